# revision 31
# baseline (speedup 1.0000x reference)
"""Trainium2 Bass kernel for the AdaptiveSNN problem.

Strategy (data parallel: batch 16384 -> 2048/core across 8 NeuronCores):
  - Block-streamed start: the 2048-col batch is a ladder of 4 column
    blocks (256/512/512/768) whose xt DMAs stream in that order (w1t+b1
    first, cold constants after block B). Each block's 7 fp32 K-tile
    matmuls (cur1 = W1 @ x^T, +b1 via a ScalarE Identity copy) pipeline
    behind its own DMA, so the DVE starts ~17.5us instead of waiting
    ~45us for the full 6.4MB x DMA.
  - Layer-1 LIF is the fused custom DVE op (1 elem/lane/cycle @0.96GHz —
    the DVE is the pacing engine; its total work is the runtime wall).
    Membrane/sign tiles are full-width per STEP, written in per-block
    slices during the fill phase, which is emitted as one consecutive
    RUN per block (the engines' 4-deep wait queues then let each block's
    serial chain execute at its own ~0.5-1.1us/step latency as soon as
    its cur1 lands, instead of head-of-line blocking on later blocks).
    All layer-2 work rides with the last block's run. From step MERGE_T
    on, all blocks have caught up and each step is ONE full-width LIF +
    ONE full-width Sign at ~2.43us/step, the DVE roofline.
  - Spikes are Sign(m-1) (+-1 bf16) on the Scalar engine; layer-2 chunk
    matmuls use 0.5*W2 hi/lo bf16 moving + a K=1 constant-row matmul
    (exact to ~1e-7). Layer-2 LIF ([128,160]) is one lagged custom DVE op
    reading cur2 from PSUM; only mem2 is DMA'd out and the host
    recomputes spk2 = (mem2 > 1) bit-exactly.
"""
import numpy as np
import ml_dtypes

import concourse.bacc as bacc
import concourse.mybir as mybir
import concourse.tile as tile
from concourse.tile import add_dep_helper
import concourse.dve_ops as dve_ops
from concourse.dve_spec import Spec, Src0, Src1, C0, C1
from concourse.dve_ops import DveOp
from concourse.bass_utils import run_bass_kernel_spmd

F32 = mybir.dt.float32
BF16 = mybir.dt.bfloat16
Alu = mybir.AluOpType

N_CORES = 8
B_FULL = 16384
B = B_FULL // N_CORES          # 2048 batch rows per core
D_IN = 784                     # 28*28
H1 = 128
H2 = 10
STEPS = 25
KT = 112                       # K-tile size: 784 = 7 * 112
NKT = D_IN // KT
NCHUNK = B // 128              # 16 batch chunks per core
THRESH = 1.0

# Column-block ladder (name, col0, width): small first block -> early start.
BLOCKS = [
    ("A", 0,    256),
    ("B", 256,  512),
    ("C", 768,  512),
    ("D", 1280, 768),
]
DMA_ORDER = ["A", "B", "C", "D"]
MERGE_T = 10      # steps >= this use single full-width LIF/Sign ops
LAG = 2           # lif2(s) trails the step-s chunk matmuls by 2 steps


def _register_lif():
    """Custom DVE op: out = ((in0*s0 + in1) - (in0 > s1))."""
    if "LIF_STEP_ANT" in dve_ops._SUB_OPCODE_FOR_NAME:
        return next(op for op in dve_ops.OPS if op.name == "LIF_STEP_ANT")
    op = DveOp(
        "LIF_STEP_ANT",
        Spec(
            body=(Src0 * C0 + Src1) - (Src0 > C1),
            reference=lambda in0, in1, s0, s1, imm2: (
                (in0 * s0 + in1) - (in0 > s1).astype(np.float32)
            ),
        ),
        subdim=False,
        uops_sha={"v3": "4d971942aba05d49", "v4": "da6677450a1cb1b9"},
    )
    dve_ops.OPS.append(op)
    dve_ops._SUB_OPCODE_FOR_NAME[op.name] = (
        dve_ops._CUSTOM_DVE_ROW_BASE + len(dve_ops.OPS) - 1
    )
    dve_ops.CUSTOM_DVE_SPECS[op.name] = op.spec
    return op


_GRAPH_CACHE = {}


def _build_graph(beta1: float, beta2: float):
    key = (beta1, beta2)
    if key in _GRAPH_CACHE:
        return _GRAPH_CACHE[key]
    LIF = _register_lif()
    Sign = mybir.ActivationFunctionType.Sign
    Ident = mybir.ActivationFunctionType.Identity

    nc = bacc.Bacc("TRN2", target_bir_lowering=False, debug=False,
                   num_devices=N_CORES)

    xt_d = nc.dram_tensor("xt", [NKT, KT, B], F32, kind="ExternalInput").ap()
    w1t_d = nc.dram_tensor("w1t", [KT, NKT * H1], F32, kind="ExternalInput").ap()
    b1_d = nc.dram_tensor("b1", [H1, 1], F32, kind="ExternalInput").ap()
    w2h_d = nc.dram_tensor("w2h", [H1, H2], BF16, kind="ExternalInput").ap()
    w2l_d = nc.dram_tensor("w2l", [H1, H2], BF16, kind="ExternalInput").ap()
    cc_d = nc.dram_tensor("cc160", [1, NCHUNK * H2], F32, kind="ExternalInput").ap()

    out_mem = nc.dram_tensor("out_mem", [STEPS, 128, NCHUNK * H2], F32,
                             kind="ExternalOutput").ap()

    with tile.TileContext(nc) as tc:
        with tc.tile_pool(name="const", bufs=1) as cpool, \
             tc.tile_pool(name="xin", bufs=1) as xpool, \
             tc.tile_pool(name="m1p", bufs=10) as m1pool, \
             tc.tile_pool(name="m2p", bufs=4) as m2pool, \
             tc.tile_pool(name="sgp", bufs=11) as sgpool, \
             tc.tile_pool(name="psw", bufs=1, space="PSUM") as pswarm, \
             tc.tile_pool(name="ps", bufs=7, space="PSUM") as pspool:

            # preload ACT tables (Sign + Identity) before anything else
            warm_t = cpool.tile([H1, 1], F32, tag="warm")
            nc.scalar.activation(warm_t[:], nc.const_aps.tensor(0.0, (H1, 1)),
                                 Sign, bias=0.0)
            nc.scalar.activation(warm_t[:], nc.const_aps.tensor(0.0, (H1, 1)),
                                 Ident, bias=0.0)

            # ---- DMA order: w1t+b1 (needed first), A/B xt, the cold
            # constants (first used ~25us in), then C/D xt ----
            ones_t = cpool.tile([1, H1], F32, tag="ones")
            nc.vector.memset(ones_t[:], 1.0)
            neg1_t = cpool.tile([H1, 1], F32, tag="neg1")
            nc.vector.memset(neg1_t[:], -1.0)
            zeros_t = cpool.tile([KT, 512], F32, tag="zeros")
            nc.vector.memset(zeros_t[:], 0.0)
            w1t_all = cpool.tile([KT, NKT * H1], F32, tag="w1t")
            nc.sync.dma_start(w1t_all[:], w1t_d)
            w1t_tiles = [w1t_all[:, k * H1:(k + 1) * H1] for k in range(NKT)]
            b1_t = cpool.tile([H1, 1], F32, tag="b1")
            nc.sync.dma_start(b1_t[:], b1_d)

            # xt tiles: rotating pools shared by (A,C) and (B,D). C's DMA
            # reuses A's buffers so it naturally waits for A's matmul
            # readers (done long before C's data could arrive anyway), and
            # likewise D after B — sequencing the stream without explicit
            # completion-semaphore chains. A and B stream concurrently from
            # the start; all DMAs go down the single Sync queue.
            blkmap = {bn: (c0, w) for (bn, c0, w) in BLOCKS}
            PAIR_W = {"A": 512, "C": 512, "B": 768, "D": 768}
            PAIR_TAG = {"A": "ac", "C": "ac", "B": "bd", "D": "bd"}
            xt_tiles = {}

            def emit_xt(bn):
                c0, w = blkmap[bn]
                for k in range(NKT):
                    xt = xpool.tile([KT, PAIR_W[bn]], F32,
                                    tag=f"xt_{PAIR_TAG[bn]}{k}",
                                    name=f"xt_{bn}{k}")[:, :w]
                    xt_tiles[(bn, k)] = xt
                    nc.sync.dma_start(xt[:], xt_d[k][:, c0:c0 + w])

            emit_xt("A")
            emit_xt("B")
            w2h_t = cpool.tile([H1, H2], BF16, tag="w2h")
            nc.sync.dma_start(w2h_t[:], w2h_d)
            w2l_t = cpool.tile([H1, H2], BF16, tag="w2l")
            nc.sync.dma_start(w2l_t[:], w2l_d)
            cc_t = cpool.tile([1, NCHUNK * H2], F32, tag="cc160")
            nc.sync.dma_start(cc_t[:], cc_d)
            emit_xt("C")
            emit_xt("D")

            # ---- cur1 per block: 7 fp32 K-matmuls -> psum -> sbuf(+b1) ----
            # HAM warm-up: zero matmuls into block A's psum while its DMA
            # streams, so the PE is at full clock for the real fp32 work.
            cur1_t = cpool.tile([H1, B], F32, tag="cur1")
            ps_warm = pswarm.tile([128, 512], F32, tag="psw", name="ps_warm")

            def fillers(n):
                # Zero matmuls with no data deps: keep the PE's HAM clock
                # ramped while it would otherwise idle waiting for the next
                # block's DMA (a cold PE runs fp32 matmuls at half speed).
                for _ in range(n):
                    nc.tensor.matmul(ps_warm[:], zeros_t[:, :128], zeros_t[:],
                                     start=True, stop=True)

            copy_jobs = {}   # block name -> list of (psum_ap, cur1_slice)
            FILLERS = {"A": 5, "B": 0, "C": 0, "D": 0}
            for bn in DMA_ORDER:   # PE queue order must match DMA arrival
                c0, w = blkmap[bn]
                fillers(FILLERS[bn])
                jobs = []
                off = c0
                rem = w
                while rem > 0:
                    pw = min(rem, 512)
                    ps = pspool.tile([128, 512], F32, tag="ps",
                                     name=f"cur1_{bn}_{off}")[:, :pw]
                    for k in range(NKT):
                        nc.tensor.matmul(ps, w1t_tiles[k],
                                         xt_tiles[(bn, k)][:, off - c0:
                                                           off - c0 + pw],
                                         start=(k == 0), stop=(k == NKT - 1))
                    jobs.append((ps, cur1_t[:, off:off + pw]))
                    off += pw
                    rem -= pw
                copy_jobs[bn] = jobs

            def emit_copy(bn):
                for ps, dst in copy_jobs[bn]:
                    # psum -> sbuf with +b1 per-partition bias on ScalarE
                    nc.scalar.activation(dst, ps, Ident, bias=b1_t[:])


            # ---- state: full-width per-step tiles, slice-written ----
            m1_tiles = {}       # t -> [128, 2048] membrane tile
            sg_tiles = {}       # t -> [128, 2048] bf16 sign tile
            m2_cur = None       # layer-2 membrane [128, 160]
            ps2 = {}            # s -> psum slice [128, 160]
            nmm = {}            # s -> chunk-matmul pairs emitted so far

            def m1_in(t, c0, w):
                # membrane state entering step t's update (t>=1): step t-1's
                # output; step 0's "update" is free (m1(0) = cur1).
                if t == 1:
                    return cur1_t[:, c0:c0 + w]
                return m1_tiles[t - 1][:, c0:c0 + w]

            def get_m1(t):
                if t not in m1_tiles:
                    m1_tiles[t] = m1pool.tile([H1, B], F32, tag="m1",
                                              name=f"m1_{t}")
                return m1_tiles[t]

            def get_sg(t):
                if t not in sg_tiles:
                    sg_tiles[t] = sgpool.tile([H1, B], BF16, tag="sg",
                                              name=f"sg_{t}")
                return sg_tiles[t]

            def emit_lif1(t, c0, w):
                out = get_m1(t)[:, c0:c0 + w]
                nc.vector._custom_dve(LIF, out=out, in0=m1_in(t, c0, w),
                                      in1=cur1_t[:, c0:c0 + w],
                                      s0=beta1, s1=THRESH)

            def emit_sign(t, c0, w):
                src = cur1_t[:, c0:c0 + w] if t == 0 else \
                    m1_tiles[t][:, c0:c0 + w]
                nc.scalar.activation(get_sg(t)[:, c0:c0 + w], src, Sign,
                                     bias=neg1_t[:])

            def emit_chunks(s, c0, w):
                # cc row starts the psum accumulation group for step s
                if s not in ps2:
                    p = pspool.tile([128, 512], F32, tag="ps",
                                    name=f"ps2_{s}")[:, :NCHUNK * H2]
                    nc.tensor.matmul(p, ones_t[:], cc_t[:], start=True,
                                     stop=False)
                    ps2[s] = p
                    nmm[s] = 0
                p = ps2[s]
                sgt = sg_tiles[s]
                for c in range(c0 // 128, (c0 + w) // 128):
                    o = p[:, c * H2:(c + 1) * H2]
                    sgc = sgt[:, c * 128:(c + 1) * 128]
                    nmm[s] += 1
                    nc.tensor.matmul(o, sgc, w2h_t[:], start=False, stop=False)
                    nc.tensor.matmul(o, sgc, w2l_t[:], start=False,
                                     stop=(nmm[s] == NCHUNK))

            def dve_lif2(s):
                nonlocal m2_cur
                p = ps2.pop(s)
                m2 = m2pool.tile([128, NCHUNK * H2], F32, tag="m2")
                if s == 0:
                    # m2(0) = cur2(0): beta*0 + cur2 - 0
                    nc.vector.tensor_scalar(m2[:], p, 0.0, None, Alu.add)
                else:
                    nc.vector._custom_dve(LIF, out=m2[:], in0=m2_cur[:],
                                          in1=p, s0=beta2, s1=THRESH)
                m2_cur = m2
                nc.sync.dma_start(out_mem[s], m2[:])

            # ---- fill phase: one consecutive run per block ----
            # Each engine's 4-deep wait-queue lets a block's serial chain
            # execute at its own latency (~0.73us/step) as soon as its
            # cur1 lands; runs are ordered by DMA arrival so nothing parks
            # more than one instruction deep. All layer-2 work (chunk
            # matmuls, cc row, lagged lif2) rides with the LAST block's
            # run, when every block's signs for that step already exist.
            for bn in DMA_ORDER:
                c0, w = blkmap[bn]
                emit_copy(bn)
                for t in range(MERGE_T):
                    if t >= 1:
                        emit_lif1(t, c0, w)
                    emit_sign(t, c0, w)
                    if bn == DMA_ORDER[-1]:
                        emit_chunks(t, 0, B)
                        if t >= LAG:
                            dve_lif2(t - LAG)
            # ---- merged phase: one full-width LIF + Sign per step ----
            for t in range(MERGE_T, STEPS):
                emit_lif1(t, 0, B)
                emit_sign(t, 0, B)
                emit_chunks(t, 0, B)
                dve_lif2(t - LAG)
            for s in range(STEPS - LAG, STEPS):
                dve_lif2(s)

    nc.compile()
    _GRAPH_CACHE[key] = nc
    return nc


def prepare_in_maps(x, W1, b1, W2, b2):
    x = np.asarray(x, dtype=np.float32)
    W1 = np.asarray(W1, dtype=np.float32)
    b1 = np.asarray(b1, dtype=np.float32)
    W2 = np.asarray(W2, dtype=np.float32)
    b2 = np.asarray(b2, dtype=np.float32)
    xf = x.reshape(B_FULL, D_IN)
    xT = xf.T.reshape(NKT, KT, B_FULL)                    # [7, 112, 16384]
    W1T = np.ascontiguousarray(
        W1.T.reshape(NKT, KT, H1).transpose(1, 0, 2).reshape(KT, NKT * H1))
    b1c = np.ascontiguousarray(b1.reshape(H1, 1))
    W2T_half = 0.5 * W2.T                                 # [128, 10]
    w2h = W2T_half.astype(ml_dtypes.bfloat16)
    w2l = (W2T_half - w2h.astype(np.float32)).astype(ml_dtypes.bfloat16)
    ccrow = (0.5 * W2.sum(axis=1) + b2).astype(np.float32)
    cc160 = np.ascontiguousarray(
        np.tile(ccrow, NCHUNK).reshape(1, NCHUNK * H2).astype(np.float32))
    in_maps = []
    for i in range(N_CORES):
        shard = np.ascontiguousarray(xT[:, :, i * B:(i + 1) * B])
        in_maps.append({
            "xt": shard, "w1t": W1T, "b1": b1c,
            "w2h": w2h, "w2l": w2l, "cc160": cc160,
        })
    return in_maps


def kernel(x, W1, b1, W2, b2, beta1, beta2):
    bb1 = float(np.clip(np.float32(beta1), 0.0, 1.0))
    bb2 = float(np.clip(np.float32(beta2), 0.0, 1.0))
    in_maps = prepare_in_maps(x, W1, b1, W2, b2)
    nc = _build_graph(bb1, bb2)
    res = run_bass_kernel_spmd(nc, in_maps, list(range(N_CORES)), trace=False)

    mem_parts = []
    for i in range(N_CORES):
        r = res.results[i]
        # [25, 128, 16*10] -> [25, 2048, 10]; batch = chunk*128 + partition
        mem = r["out_mem"].reshape(STEPS, 128, NCHUNK, H2)
        mem_parts.append(np.transpose(mem, (0, 2, 1, 3)).reshape(STEPS, B, H2))
    mem2 = np.ascontiguousarray(
        np.concatenate(mem_parts, axis=1).astype(np.float32))
    # spikes are a pure function of the (bit-exact) membrane values
    spk2 = (mem2 > np.float32(THRESH)).astype(np.float32)
    return spk2, mem2


# revision 32
# speedup vs baseline: 1.0419x; 1.0419x over previous
"""Trainium2 Bass kernel for the AdaptiveSNN problem.

Strategy (data parallel: batch 16384 -> 2048/core across 8 NeuronCores):
  - Block-streamed start: the 2048-col batch is a ladder of 4 column
    blocks (256/512/512/768) whose xt DMAs stream in that order (w1t+b1
    first, cold constants after block B). Each block's 7 fp32 K-tile
    matmuls (cur1 = W1 @ x^T, +b1 via a ScalarE Identity copy) pipeline
    behind its own DMA, so the DVE starts ~17.5us instead of waiting
    ~45us for the full 6.4MB x DMA.
  - Layer-1 LIF is the fused custom DVE op (1 elem/lane/cycle @0.96GHz —
    the DVE is the pacing engine; its total work is the runtime wall).
    Membrane/sign tiles are full-width per STEP, written in per-block
    slices during the fill phase, which is emitted as one consecutive
    RUN per block (the engines' 4-deep wait queues then let each block's
    serial chain execute at its own ~0.5-1.1us/step latency as soon as
    its cur1 lands, instead of head-of-line blocking on later blocks).
    All layer-2 work rides with the last block's run. From step MERGE_T
    on, all blocks have caught up and each step is ONE full-width LIF +
    ONE full-width Sign at ~2.43us/step, the DVE roofline.
  - Spikes are Sign(m-1) (+-1 bf16) on the Scalar engine; layer-2 chunk
    matmuls use 0.5*W2 hi/lo bf16 moving + a K=1 constant-row matmul
    (exact to ~1e-7). Layer-2 LIF ([128,160]) is one lagged custom DVE op
    reading cur2 from PSUM; only mem2 is DMA'd out and the host
    recomputes spk2 = (mem2 > 1) bit-exactly.
"""
import numpy as np
import ml_dtypes

import concourse.bacc as bacc
import concourse.mybir as mybir
import concourse.tile as tile
from concourse.tile import add_dep_helper
import concourse.dve_ops as dve_ops
from concourse.dve_spec import Spec, Src0, Src1, C0, C1
from concourse.dve_ops import DveOp
from concourse.bass_utils import run_bass_kernel_spmd

F32 = mybir.dt.float32
BF16 = mybir.dt.bfloat16
Alu = mybir.AluOpType

N_CORES = 8
B_FULL = 16384
B = B_FULL // N_CORES          # 2048 batch rows per core
D_IN = 784                     # 28*28
H1 = 128
H2 = 10
STEPS = 25
KT = 112                       # K-tile size: 784 = 7 * 112
NKT = D_IN // KT
NCHUNK = B // 128              # 16 batch chunks per core
THRESH = 1.0

# Column-block ladder (name, col0, width): small first block -> early start.
BLOCKS = [
    ("A", 0,    256),
    ("B", 256,  512),
    ("C", 768,  512),
    ("D", 1280, 768),
]
DMA_ORDER = ["A", "B", "C", "D"]
MERGE_T = 10      # steps >= this use single full-width LIF/Sign ops
LAG = 2           # lif2(s) trails the step-s chunk matmuls by 2 steps


def _register_lif():
    """Custom DVE op: out = ((in0*s0 + in1) - (in0 > s1))."""
    if "LIF_STEP_ANT" in dve_ops._SUB_OPCODE_FOR_NAME:
        return next(op for op in dve_ops.OPS if op.name == "LIF_STEP_ANT")
    op = DveOp(
        "LIF_STEP_ANT",
        Spec(
            body=(Src0 * C0 + Src1) - (Src0 > C1),
            reference=lambda in0, in1, s0, s1, imm2: (
                (in0 * s0 + in1) - (in0 > s1).astype(np.float32)
            ),
        ),
        subdim=False,
        uops_sha={"v3": "4d971942aba05d49", "v4": "da6677450a1cb1b9"},
    )
    dve_ops.OPS.append(op)
    dve_ops._SUB_OPCODE_FOR_NAME[op.name] = (
        dve_ops._CUSTOM_DVE_ROW_BASE + len(dve_ops.OPS) - 1
    )
    dve_ops.CUSTOM_DVE_SPECS[op.name] = op.spec
    return op


_GRAPH_CACHE = {}


def _build_graph(beta1: float, beta2: float):
    key = (beta1, beta2)
    if key in _GRAPH_CACHE:
        return _GRAPH_CACHE[key]
    LIF = _register_lif()
    Sign = mybir.ActivationFunctionType.Sign
    Ident = mybir.ActivationFunctionType.Identity

    nc = bacc.Bacc("TRN2", target_bir_lowering=False, debug=False,
                   num_devices=N_CORES)

    xt_d = nc.dram_tensor("xt", [NKT, KT, B], F32, kind="ExternalInput").ap()
    w1t_d = nc.dram_tensor("w1t", [KT, NKT * H1], F32, kind="ExternalInput").ap()
    b1_d = nc.dram_tensor("b1", [H1, 1], F32, kind="ExternalInput").ap()
    w2h_d = nc.dram_tensor("w2h", [H1, H2], BF16, kind="ExternalInput").ap()
    w2l_d = nc.dram_tensor("w2l", [H1, H2], BF16, kind="ExternalInput").ap()
    cc_d = nc.dram_tensor("cc160", [1, NCHUNK * H2], F32, kind="ExternalInput").ap()

    out_mem = nc.dram_tensor("out_mem", [STEPS, 128, NCHUNK * H2], F32,
                             kind="ExternalOutput").ap()

    with tile.TileContext(nc) as tc:
        with tc.tile_pool(name="const", bufs=1) as cpool, \
             tc.tile_pool(name="xin", bufs=1) as xpool, \
             tc.tile_pool(name="m1p", bufs=10) as m1pool, \
             tc.tile_pool(name="m2p", bufs=4) as m2pool, \
             tc.tile_pool(name="sgp", bufs=11) as sgpool, \
             tc.tile_pool(name="psw", bufs=1, space="PSUM") as pswarm, \
             tc.tile_pool(name="ps", bufs=7, space="PSUM") as pspool:

            # preload ACT tables (Sign + Identity) before anything else
            warm_t = cpool.tile([H1, 1], F32, tag="warm")
            nc.scalar.activation(warm_t[:], nc.const_aps.tensor(0.0, (H1, 1)),
                                 Sign, bias=0.0)
            nc.scalar.activation(warm_t[:], nc.const_aps.tensor(0.0, (H1, 1)),
                                 Ident, bias=0.0)

            # ---- DMA order: w1t+b1 (needed first), A/B xt, the cold
            # constants (first used ~25us in), then C/D xt ----
            ones_t = cpool.tile([1, H1], F32, tag="ones")
            nc.vector.memset(ones_t[:], 1.0)
            neg1_t = cpool.tile([H1, 1], F32, tag="neg1")
            nc.vector.memset(neg1_t[:], -1.0)
            zeros_t = cpool.tile([KT, 512], F32, tag="zeros")
            nc.vector.memset(zeros_t[:], 0.0)
            w1t_all = cpool.tile([KT, NKT * H1], F32, tag="w1t")
            nc.sync.dma_start(w1t_all[:], w1t_d)
            w1t_tiles = [w1t_all[:, k * H1:(k + 1) * H1] for k in range(NKT)]
            b1_t = cpool.tile([H1, 1], F32, tag="b1")
            nc.sync.dma_start(b1_t[:], b1_d)

            # xt tiles: rotating pools shared by (A,C) and (B,D). C's DMA
            # reuses A's buffers so it naturally waits for A's matmul
            # readers (done long before C's data could arrive anyway), and
            # likewise D after B — sequencing the stream without explicit
            # completion-semaphore chains. A and B stream concurrently from
            # the start; all DMAs go down the single Sync queue.
            blkmap = {bn: (c0, w) for (bn, c0, w) in BLOCKS}
            PAIR_W = {"A": 512, "C": 512, "B": 768, "D": 768}
            PAIR_TAG = {"A": "ac", "C": "ac", "B": "bd", "D": "bd"}
            xt_tiles = {}

            def emit_xt(bn):
                c0, w = blkmap[bn]
                for k in range(NKT):
                    xt = xpool.tile([KT, PAIR_W[bn]], F32,
                                    tag=f"xt_{PAIR_TAG[bn]}{k}",
                                    name=f"xt_{bn}{k}")[:, :w]
                    xt_tiles[(bn, k)] = xt
                    nc.sync.dma_start(xt[:], xt_d[k][:, c0:c0 + w])

            emit_xt("A")
            emit_xt("B")
            w2h_t = cpool.tile([H1, H2], BF16, tag="w2h")
            nc.sync.dma_start(w2h_t[:], w2h_d)
            w2l_t = cpool.tile([H1, H2], BF16, tag="w2l")
            nc.sync.dma_start(w2l_t[:], w2l_d)
            cc_t = cpool.tile([1, NCHUNK * H2], F32, tag="cc160")
            nc.sync.dma_start(cc_t[:], cc_d)
            emit_xt("C")
            emit_xt("D")

            # ---- cur1 per block: 7 fp32 K-matmuls -> psum -> sbuf(+b1) ----
            # HAM warm-up: zero matmuls into block A's psum while its DMA
            # streams, so the PE is at full clock for the real fp32 work.
            cur1_t = cpool.tile([H1, B], F32, tag="cur1")
            ps_warm = pswarm.tile([128, 512], F32, tag="psw", name="ps_warm")

            def fillers(n):
                # Zero matmuls with no data deps: keep the PE's HAM clock
                # ramped while it would otherwise idle waiting for the next
                # block's DMA (a cold PE runs fp32 matmuls at half speed).
                for _ in range(n):
                    nc.tensor.matmul(ps_warm[:], zeros_t[:, :128], zeros_t[:],
                                     start=True, stop=True)

            copy_jobs = {}   # block name -> list of (psum_ap, cur1_slice)
            FILLERS = {"A": 5, "B": 0, "C": 0, "D": 0}
            for bn in DMA_ORDER:   # PE queue order must match DMA arrival
                c0, w = blkmap[bn]
                fillers(FILLERS[bn])
                jobs = []
                off = c0
                rem = w
                while rem > 0:
                    pw = min(rem, 512)
                    ps = pspool.tile([128, 512], F32, tag="ps",
                                     name=f"cur1_{bn}_{off}")[:, :pw]
                    for k in range(NKT):
                        nc.tensor.matmul(ps, w1t_tiles[k],
                                         xt_tiles[(bn, k)][:, off - c0:
                                                           off - c0 + pw],
                                         start=(k == 0), stop=(k == NKT - 1))
                    jobs.append((ps, cur1_t[:, off:off + pw]))
                    off += pw
                    rem -= pw
                copy_jobs[bn] = jobs

            def emit_copy(bn):
                for ps, dst in copy_jobs[bn]:
                    # psum -> sbuf with +b1 per-partition bias on ScalarE
                    nc.scalar.activation(dst, ps, Ident, bias=b1_t[:])


            # ---- state: full-width per-step tiles, slice-written ----
            m1_tiles = {}       # t -> [128, 2048] membrane tile
            sg_tiles = {}       # t -> [128, 2048] bf16 sign tile
            m2_cur = None       # layer-2 membrane [128, 160]
            ps2 = {}            # s -> psum slice [128, 160]
            nmm = {}            # s -> chunk-matmul pairs emitted so far

            def m1_in(t, c0, w):
                # membrane state entering step t's update (t>=1): step t-1's
                # output; step 0's "update" is free (m1(0) = cur1).
                if t == 1:
                    return cur1_t[:, c0:c0 + w]
                return m1_tiles[t - 1][:, c0:c0 + w]

            def get_m1(t):
                if t not in m1_tiles:
                    m1_tiles[t] = m1pool.tile([H1, B], F32, tag="m1",
                                              name=f"m1_{t}")
                return m1_tiles[t]

            def get_sg(t):
                if t not in sg_tiles:
                    sg_tiles[t] = sgpool.tile([H1, B], BF16, tag="sg",
                                              name=f"sg_{t}")
                return sg_tiles[t]

            def emit_lif1(t, c0, w):
                out = get_m1(t)[:, c0:c0 + w]
                nc.vector._custom_dve(LIF, out=out, in0=m1_in(t, c0, w),
                                      in1=cur1_t[:, c0:c0 + w],
                                      s0=beta1, s1=THRESH)

            def emit_sign(t, c0, w):
                src = cur1_t[:, c0:c0 + w] if t == 0 else \
                    m1_tiles[t][:, c0:c0 + w]
                nc.scalar.activation(get_sg(t)[:, c0:c0 + w], src, Sign,
                                     bias=neg1_t[:])

            def emit_chunks(s, c0, w):
                # cc row starts the psum accumulation group for step s
                if s not in ps2:
                    p = pspool.tile([128, 512], F32, tag="ps",
                                    name=f"ps2_{s}")[:, :NCHUNK * H2]
                    nc.tensor.matmul(p, ones_t[:], cc_t[:], start=True,
                                     stop=False)
                    ps2[s] = p
                    nmm[s] = 0
                p = ps2[s]
                sgt = sg_tiles[s]
                for c in range(c0 // 128, (c0 + w) // 128):
                    o = p[:, c * H2:(c + 1) * H2]
                    sgc = sgt[:, c * 128:(c + 1) * 128]
                    nmm[s] += 1
                    nc.tensor.matmul(o, sgc, w2h_t[:], start=False, stop=False)
                    nc.tensor.matmul(o, sgc, w2l_t[:], start=False,
                                     stop=(nmm[s] == NCHUNK))

            def dve_lif2(s):
                nonlocal m2_cur
                p = ps2.pop(s)
                m2 = m2pool.tile([128, NCHUNK * H2], F32, tag="m2")
                if s == 0:
                    # m2(0) = cur2(0): beta*0 + cur2 - 0
                    nc.vector.tensor_scalar(m2[:], p, 0.0, None, Alu.add)
                else:
                    nc.vector._custom_dve(LIF, out=m2[:], in0=m2_cur[:],
                                          in1=p, s0=beta2, s1=THRESH)
                m2_cur = m2
                nc.sync.dma_start(out_mem[s], m2[:])

            # PE clock pre-ramp for the chunk-matmul stream: these run in
            # the natural PE idle window after D's cur1 matmuls, before the
            # first sign-gated chunk matmuls can execute.
            fillers(10)

            # ---- fill phase: one consecutive run per block ----
            # Each engine's 4-deep wait-queue lets a block's serial chain
            # execute at its own latency (~0.73us/step) as soon as its
            # cur1 lands; runs are ordered by DMA arrival so nothing parks
            # more than one instruction deep. All layer-2 work (chunk
            # matmuls, cc row, lagged lif2) rides with the LAST block's
            # run, when every block's signs for that step already exist.
            for bn in DMA_ORDER:
                c0, w = blkmap[bn]
                emit_copy(bn)
                for t in range(MERGE_T):
                    if t >= 1:
                        emit_lif1(t, c0, w)
                    emit_sign(t, c0, w)
                    if bn == DMA_ORDER[-1]:
                        emit_chunks(t, 0, B)
                        if t >= LAG:
                            dve_lif2(t - LAG)
            # ---- merged phase: one full-width LIF + Sign per step ----
            for t in range(MERGE_T, STEPS - 1):
                emit_lif1(t, 0, B)
                emit_sign(t, 0, B)
                emit_chunks(t, 0, B)
                dve_lif2(t - LAG)
            # final step: 4 column-group pipelines (lif/sign/chunks overlap)
            # so the drain chain is ~one group long instead of full-width
            t = STEPS - 1
            for g0 in range(0, B, 512):
                emit_lif1(t, g0, 512)
                emit_sign(t, g0, 512)
                emit_chunks(t, g0, 512)
            for s in range(STEPS - 1 - LAG, STEPS):
                dve_lif2(s)

    nc.compile()
    _GRAPH_CACHE[key] = nc
    return nc


def prepare_in_maps(x, W1, b1, W2, b2):
    x = np.asarray(x, dtype=np.float32)
    W1 = np.asarray(W1, dtype=np.float32)
    b1 = np.asarray(b1, dtype=np.float32)
    W2 = np.asarray(W2, dtype=np.float32)
    b2 = np.asarray(b2, dtype=np.float32)
    xf = x.reshape(B_FULL, D_IN)
    xT = xf.T.reshape(NKT, KT, B_FULL)                    # [7, 112, 16384]
    W1T = np.ascontiguousarray(
        W1.T.reshape(NKT, KT, H1).transpose(1, 0, 2).reshape(KT, NKT * H1))
    b1c = np.ascontiguousarray(b1.reshape(H1, 1))
    W2T_half = 0.5 * W2.T                                 # [128, 10]
    w2h = W2T_half.astype(ml_dtypes.bfloat16)
    w2l = (W2T_half - w2h.astype(np.float32)).astype(ml_dtypes.bfloat16)
    ccrow = (0.5 * W2.sum(axis=1) + b2).astype(np.float32)
    cc160 = np.ascontiguousarray(
        np.tile(ccrow, NCHUNK).reshape(1, NCHUNK * H2).astype(np.float32))
    in_maps = []
    for i in range(N_CORES):
        shard = np.ascontiguousarray(xT[:, :, i * B:(i + 1) * B])
        in_maps.append({
            "xt": shard, "w1t": W1T, "b1": b1c,
            "w2h": w2h, "w2l": w2l, "cc160": cc160,
        })
    return in_maps


def kernel(x, W1, b1, W2, b2, beta1, beta2):
    bb1 = float(np.clip(np.float32(beta1), 0.0, 1.0))
    bb2 = float(np.clip(np.float32(beta2), 0.0, 1.0))
    in_maps = prepare_in_maps(x, W1, b1, W2, b2)
    nc = _build_graph(bb1, bb2)
    res = run_bass_kernel_spmd(nc, in_maps, list(range(N_CORES)), trace=False)

    mem_parts = []
    for i in range(N_CORES):
        r = res.results[i]
        # [25, 128, 16*10] -> [25, 2048, 10]; batch = chunk*128 + partition
        mem = r["out_mem"].reshape(STEPS, 128, NCHUNK, H2)
        mem_parts.append(np.transpose(mem, (0, 2, 1, 3)).reshape(STEPS, B, H2))
    mem2 = np.ascontiguousarray(
        np.concatenate(mem_parts, axis=1).astype(np.float32))
    # spikes are a pure function of the (bit-exact) membrane values
    spk2 = (mem2 > np.float32(THRESH)).astype(np.float32)
    return spk2, mem2


# revision 33
# speedup vs baseline: 1.0679x; 1.0249x over previous
"""Trainium2 Bass kernel for the AdaptiveSNN problem.

Strategy (data parallel: batch 16384 -> 2048/core across 8 NeuronCores):
  - Block-streamed start: the 2048-col batch is a ladder of 4 column
    blocks (256/512/512/768) whose xt DMAs stream in that order (w1t+b1
    first, cold constants after block B). Each block's 7 fp32 K-tile
    matmuls (cur1 = W1 @ x^T, +b1 via a ScalarE Identity copy) pipeline
    behind its own DMA, so the DVE starts ~17.5us instead of waiting
    ~45us for the full 6.4MB x DMA.
  - Layer-1 LIF is the fused custom DVE op (1 elem/lane/cycle @0.96GHz —
    the DVE is the pacing engine; its total work is the runtime wall).
    Membrane/sign tiles are full-width per STEP, written in per-block
    slices during the fill phase, which is emitted as one consecutive
    RUN per block (the engines' 4-deep wait queues then let each block's
    serial chain execute at its own ~0.5-1.1us/step latency as soon as
    its cur1 lands, instead of head-of-line blocking on later blocks).
    All layer-2 work rides with the last block's run. From step MERGE_T
    on, all blocks have caught up and each step is ONE full-width LIF +
    ONE full-width Sign at ~2.43us/step, the DVE roofline.
  - Spikes are Sign(m-1) (+-1 bf16) on the Scalar engine; layer-2 chunk
    matmuls use 0.5*W2 hi/lo bf16 moving + a K=1 constant-row matmul
    (exact to ~1e-7). Layer-2 LIF ([128,160]) is one lagged custom DVE op
    reading cur2 from PSUM; only mem2 is DMA'd out and the host
    recomputes spk2 = (mem2 > 1) bit-exactly.
"""
import numpy as np
import ml_dtypes

import concourse.bacc as bacc
import concourse.mybir as mybir
import concourse.tile as tile
from concourse.tile import add_dep_helper
import concourse.dve_ops as dve_ops
from concourse.dve_spec import Spec, Src0, Src1, C0, C1
from concourse.dve_ops import DveOp
from concourse.bass_utils import run_bass_kernel_spmd

F32 = mybir.dt.float32
BF16 = mybir.dt.bfloat16
Alu = mybir.AluOpType

N_CORES = 8
B_FULL = 16384
B = B_FULL // N_CORES          # 2048 batch rows per core
D_IN = 784                     # 28*28
H1 = 128
H2 = 10
STEPS = 25
KT = 112                       # K-tile size: 784 = 7 * 112
NKT = D_IN // KT
NCHUNK = B // 128              # 16 batch chunks per core
THRESH = 1.0

# Column-block ladder (name, col0, width): small first block -> early start.
BLOCKS = [
    ("A", 0,    256),
    ("B", 256,  512),
    ("C", 768,  512),
    ("D", 1280, 768),
]
DMA_ORDER = ["A", "B", "C", "D"]
MERGE_T = 10      # steps >= this use single full-width LIF/Sign ops
LAG = 2           # lif2(s) trails the step-s chunk matmuls by 2 steps


def _register_lif():
    """Custom DVE op: out = ((in0*s0 + in1) - (in0 > s1))."""
    if "LIF_STEP_ANT" in dve_ops._SUB_OPCODE_FOR_NAME:
        return next(op for op in dve_ops.OPS if op.name == "LIF_STEP_ANT")
    op = DveOp(
        "LIF_STEP_ANT",
        Spec(
            body=(Src0 * C0 + Src1) - (Src0 > C1),
            reference=lambda in0, in1, s0, s1, imm2: (
                (in0 * s0 + in1) - (in0 > s1).astype(np.float32)
            ),
        ),
        subdim=False,
        uops_sha={"v3": "4d971942aba05d49", "v4": "da6677450a1cb1b9"},
    )
    dve_ops.OPS.append(op)
    dve_ops._SUB_OPCODE_FOR_NAME[op.name] = (
        dve_ops._CUSTOM_DVE_ROW_BASE + len(dve_ops.OPS) - 1
    )
    dve_ops.CUSTOM_DVE_SPECS[op.name] = op.spec
    return op


_GRAPH_CACHE = {}


def _build_graph(beta1: float, beta2: float):
    key = (beta1, beta2)
    if key in _GRAPH_CACHE:
        return _GRAPH_CACHE[key]
    LIF = _register_lif()
    Sign = mybir.ActivationFunctionType.Sign
    Ident = mybir.ActivationFunctionType.Identity

    nc = bacc.Bacc("TRN2", target_bir_lowering=False, debug=False,
                   num_devices=N_CORES)

    xt_d = nc.dram_tensor("xt", [NKT, KT, B], F32, kind="ExternalInput").ap()
    w1t_d = nc.dram_tensor("w1t", [KT, NKT * H1], F32, kind="ExternalInput").ap()
    b1_d = nc.dram_tensor("b1", [H1, 1], F32, kind="ExternalInput").ap()
    w2h_d = nc.dram_tensor("w2h", [H1, H2], BF16, kind="ExternalInput").ap()
    w2l_d = nc.dram_tensor("w2l", [H1, H2], BF16, kind="ExternalInput").ap()
    cc_d = nc.dram_tensor("cc160", [1, NCHUNK * H2], F32, kind="ExternalInput").ap()

    out_mem = nc.dram_tensor("out_mem", [STEPS, 128, NCHUNK * H2], F32,
                             kind="ExternalOutput").ap()

    with tile.TileContext(nc) as tc:
        with tc.tile_pool(name="const", bufs=1) as cpool, \
             tc.tile_pool(name="xin", bufs=1) as xpool, \
             tc.tile_pool(name="m1p", bufs=10) as m1pool, \
             tc.tile_pool(name="m2p", bufs=12) as m2pool, \
             tc.tile_pool(name="sgp", bufs=11) as sgpool, \
             tc.tile_pool(name="psw", bufs=1, space="PSUM") as pswarm, \
             tc.tile_pool(name="ps", bufs=7, space="PSUM") as pspool:

            # preload ACT tables (Sign + Identity) before anything else
            warm_t = cpool.tile([H1, 1], F32, tag="warm")
            nc.scalar.activation(warm_t[:], nc.const_aps.tensor(0.0, (H1, 1)),
                                 Sign, bias=0.0)
            nc.scalar.activation(warm_t[:], nc.const_aps.tensor(0.0, (H1, 1)),
                                 Ident, bias=0.0)

            # ---- DMA order: w1t+b1 (needed first), A/B xt, the cold
            # constants (first used ~25us in), then C/D xt ----
            ones_t = cpool.tile([1, H1], F32, tag="ones")
            nc.vector.memset(ones_t[:], 1.0)
            neg1_t = cpool.tile([H1, 1], F32, tag="neg1")
            nc.vector.memset(neg1_t[:], -1.0)
            zeros_t = cpool.tile([KT, 512], F32, tag="zeros")
            nc.vector.memset(zeros_t[:], 0.0)
            w1t_all = cpool.tile([KT, NKT * H1], F32, tag="w1t")
            nc.sync.dma_start(w1t_all[:], w1t_d)
            w1t_tiles = [w1t_all[:, k * H1:(k + 1) * H1] for k in range(NKT)]
            b1_t = cpool.tile([H1, 1], F32, tag="b1")
            nc.sync.dma_start(b1_t[:], b1_d)

            # xt tiles: rotating pools shared by (A,C) and (B,D). C's DMA
            # reuses A's buffers so it naturally waits for A's matmul
            # readers (done long before C's data could arrive anyway), and
            # likewise D after B — sequencing the stream without explicit
            # completion-semaphore chains. A and B stream concurrently from
            # the start; all DMAs go down the single Sync queue.
            blkmap = {bn: (c0, w) for (bn, c0, w) in BLOCKS}
            PAIR_W = {"A": 512, "C": 512, "B": 768, "D": 768}
            PAIR_TAG = {"A": "ac", "C": "ac", "B": "bd", "D": "bd"}
            xt_tiles = {}

            def emit_xt(bn):
                c0, w = blkmap[bn]
                for k in range(NKT):
                    xt = xpool.tile([KT, PAIR_W[bn]], F32,
                                    tag=f"xt_{PAIR_TAG[bn]}{k}",
                                    name=f"xt_{bn}{k}")[:, :w]
                    xt_tiles[(bn, k)] = xt
                    nc.sync.dma_start(xt[:], xt_d[k][:, c0:c0 + w])

            emit_xt("A")
            emit_xt("B")
            w2h_t = cpool.tile([H1, H2], BF16, tag="w2h")
            nc.sync.dma_start(w2h_t[:], w2h_d)
            w2l_t = cpool.tile([H1, H2], BF16, tag="w2l")
            nc.sync.dma_start(w2l_t[:], w2l_d)
            cc_t = cpool.tile([1, NCHUNK * H2], F32, tag="cc160")
            nc.sync.dma_start(cc_t[:], cc_d)
            emit_xt("C")
            emit_xt("D")

            # ---- cur1 per block: 7 fp32 K-matmuls -> psum -> sbuf(+b1) ----
            # HAM warm-up: zero matmuls into block A's psum while its DMA
            # streams, so the PE is at full clock for the real fp32 work.
            cur1_t = cpool.tile([H1, B], F32, tag="cur1")
            ps_warm = pswarm.tile([128, 512], F32, tag="psw", name="ps_warm")

            def fillers(n):
                # Zero matmuls with no data deps: keep the PE's HAM clock
                # ramped while it would otherwise idle waiting for the next
                # block's DMA (a cold PE runs fp32 matmuls at half speed).
                for _ in range(n):
                    nc.tensor.matmul(ps_warm[:], zeros_t[:, :128], zeros_t[:],
                                     start=True, stop=True)

            copy_jobs = {}   # block name -> list of (psum_ap, cur1_slice)
            FILLERS = {"A": 5, "B": 0, "C": 0, "D": 0}
            for bn in DMA_ORDER:   # PE queue order must match DMA arrival
                c0, w = blkmap[bn]
                fillers(FILLERS[bn])
                jobs = []
                off = c0
                rem = w
                while rem > 0:
                    pw = min(rem, 512)
                    ps = pspool.tile([128, 512], F32, tag="ps",
                                     name=f"cur1_{bn}_{off}")[:, :pw]
                    for k in range(NKT):
                        nc.tensor.matmul(ps, w1t_tiles[k],
                                         xt_tiles[(bn, k)][:, off - c0:
                                                           off - c0 + pw],
                                         start=(k == 0), stop=(k == NKT - 1))
                    jobs.append((ps, cur1_t[:, off:off + pw]))
                    off += pw
                    rem -= pw
                copy_jobs[bn] = jobs

            def emit_copy(bn):
                for ps, dst in copy_jobs[bn]:
                    # psum -> sbuf with +b1 per-partition bias on ScalarE
                    nc.scalar.activation(dst, ps, Ident, bias=b1_t[:])


            # ---- state: full-width per-step tiles, slice-written ----
            m1_tiles = {}       # t -> [128, 2048] membrane tile
            sg_tiles = {}       # t -> [128, 2048] bf16 sign tile
            m2_cur = None       # layer-2 membrane [128, 160]
            ps2 = {}            # s -> psum slice [128, 160]
            nmm = {}            # s -> chunk-matmul pairs emitted so far

            def m1_in(t, c0, w):
                # membrane state entering step t's update (t>=1): step t-1's
                # output; step 0's "update" is free (m1(0) = cur1).
                if t == 1:
                    return cur1_t[:, c0:c0 + w]
                return m1_tiles[t - 1][:, c0:c0 + w]

            def get_m1(t):
                if t not in m1_tiles:
                    m1_tiles[t] = m1pool.tile([H1, B], F32, tag="m1",
                                              name=f"m1_{t}")
                return m1_tiles[t]

            def get_sg(t):
                if t not in sg_tiles:
                    sg_tiles[t] = sgpool.tile([H1, B], BF16, tag="sg",
                                              name=f"sg_{t}")
                return sg_tiles[t]

            def emit_lif1(t, c0, w):
                out = get_m1(t)[:, c0:c0 + w]
                nc.vector._custom_dve(LIF, out=out, in0=m1_in(t, c0, w),
                                      in1=cur1_t[:, c0:c0 + w],
                                      s0=beta1, s1=THRESH)

            def emit_sign(t, c0, w):
                src = cur1_t[:, c0:c0 + w] if t == 0 else \
                    m1_tiles[t][:, c0:c0 + w]
                nc.scalar.activation(get_sg(t)[:, c0:c0 + w], src, Sign,
                                     bias=neg1_t[:])

            def emit_chunks(s, c0, w):
                # cc row starts the psum accumulation group for step s
                if s not in ps2:
                    p = pspool.tile([128, 512], F32, tag="ps",
                                    name=f"ps2_{s}")[:, :NCHUNK * H2]
                    nc.tensor.matmul(p, ones_t[:], cc_t[:], start=True,
                                     stop=False)
                    ps2[s] = p
                    nmm[s] = 0
                p = ps2[s]
                sgt = sg_tiles[s]
                for c in range(c0 // 128, (c0 + w) // 128):
                    o = p[:, c * H2:(c + 1) * H2]
                    sgc = sgt[:, c * 128:(c + 1) * 128]
                    nmm[s] += 1
                    nc.tensor.matmul(o, sgc, w2h_t[:], start=False, stop=False)
                    nc.tensor.matmul(o, sgc, w2l_t[:], start=False,
                                     stop=(nmm[s] == NCHUNK))

            def dve_lif2(s):
                nonlocal m2_cur
                p = ps2.pop(s)
                m2 = m2pool.tile([128, NCHUNK * H2], F32, tag="m2")
                if s == 0:
                    # m2(0) = cur2(0): beta*0 + cur2 - 0
                    nc.vector.tensor_scalar(m2[:], p, 0.0, None, Alu.add)
                else:
                    nc.vector._custom_dve(LIF, out=m2[:], in0=m2_cur[:],
                                          in1=p, s0=beta2, s1=THRESH)
                m2_cur = m2
                nc.sync.dma_start(out_mem[s], m2[:])

            # PE clock pre-ramp for the chunk-matmul stream: these run in
            # the natural PE idle window after D's cur1 matmuls, before the
            # first sign-gated chunk matmuls can execute.
            fillers(8)

            # ---- fill phase: one consecutive run per block ----
            # Each engine's 4-deep wait-queue lets a block's serial chain
            # execute at its own latency (~0.73us/step) as soon as its
            # cur1 lands; runs are ordered by DMA arrival so nothing parks
            # more than one instruction deep. All layer-2 work (chunk
            # matmuls, cc row, lagged lif2) rides with the LAST block's
            # run, when every block's signs for that step already exist.
            for bn in DMA_ORDER:
                c0, w = blkmap[bn]
                emit_copy(bn)
                for t in range(MERGE_T):
                    if t >= 1:
                        emit_lif1(t, c0, w)
                    emit_sign(t, c0, w)
                    if bn == DMA_ORDER[-1]:
                        emit_chunks(t, 0, B)
                        if t >= LAG:
                            dve_lif2(t - LAG)
            # ---- merged phase: one full-width LIF + Sign per step ----
            for t in range(MERGE_T, STEPS - 1):
                emit_lif1(t, 0, B)
                emit_sign(t, 0, B)
                emit_chunks(t, 0, B)
                dve_lif2(t - LAG)
            # final step: 4 column-group pipelines (lif/sign/chunks overlap)
            # so the drain chain is ~one group long instead of full-width
            t = STEPS - 1
            for g0 in range(0, B, 512):
                emit_lif1(t, g0, 512)
                emit_sign(t, g0, 512)
                emit_chunks(t, g0, 512)
            for s in range(STEPS - 1 - LAG, STEPS):
                dve_lif2(s)

    nc.compile()
    _GRAPH_CACHE[key] = nc
    return nc


def prepare_in_maps(x, W1, b1, W2, b2):
    x = np.asarray(x, dtype=np.float32)
    W1 = np.asarray(W1, dtype=np.float32)
    b1 = np.asarray(b1, dtype=np.float32)
    W2 = np.asarray(W2, dtype=np.float32)
    b2 = np.asarray(b2, dtype=np.float32)
    xf = x.reshape(B_FULL, D_IN)
    xT = xf.T.reshape(NKT, KT, B_FULL)                    # [7, 112, 16384]
    W1T = np.ascontiguousarray(
        W1.T.reshape(NKT, KT, H1).transpose(1, 0, 2).reshape(KT, NKT * H1))
    b1c = np.ascontiguousarray(b1.reshape(H1, 1))
    W2T_half = 0.5 * W2.T                                 # [128, 10]
    w2h = W2T_half.astype(ml_dtypes.bfloat16)
    w2l = (W2T_half - w2h.astype(np.float32)).astype(ml_dtypes.bfloat16)
    ccrow = (0.5 * W2.sum(axis=1) + b2).astype(np.float32)
    cc160 = np.ascontiguousarray(
        np.tile(ccrow, NCHUNK).reshape(1, NCHUNK * H2).astype(np.float32))
    in_maps = []
    for i in range(N_CORES):
        shard = np.ascontiguousarray(xT[:, :, i * B:(i + 1) * B])
        in_maps.append({
            "xt": shard, "w1t": W1T, "b1": b1c,
            "w2h": w2h, "w2l": w2l, "cc160": cc160,
        })
    return in_maps


def kernel(x, W1, b1, W2, b2, beta1, beta2):
    bb1 = float(np.clip(np.float32(beta1), 0.0, 1.0))
    bb2 = float(np.clip(np.float32(beta2), 0.0, 1.0))
    in_maps = prepare_in_maps(x, W1, b1, W2, b2)
    nc = _build_graph(bb1, bb2)
    res = run_bass_kernel_spmd(nc, in_maps, list(range(N_CORES)), trace=False)

    mem_parts = []
    for i in range(N_CORES):
        r = res.results[i]
        # [25, 128, 16*10] -> [25, 2048, 10]; batch = chunk*128 + partition
        mem = r["out_mem"].reshape(STEPS, 128, NCHUNK, H2)
        mem_parts.append(np.transpose(mem, (0, 2, 1, 3)).reshape(STEPS, B, H2))
    mem2 = np.ascontiguousarray(
        np.concatenate(mem_parts, axis=1).astype(np.float32))
    # spikes are a pure function of the (bit-exact) membrane values
    spk2 = (mem2 > np.float32(THRESH)).astype(np.float32)
    return spk2, mem2


# revision 50
# speedup vs baseline: 1.0953x; 1.0257x over previous
"""Trainium2 Bass kernel for the AdaptiveSNN problem.

Strategy (data parallel: batch 16384 -> 2048/core across 8 NeuronCores):
  - Block-streamed start: the 2048-col batch is a ladder of 4 column
    blocks (256/512/512/768) whose xt DMAs stream in that order (w1t+b1
    first, cold constants after block B). Each block's 7 fp32 K-tile
    matmuls (cur1 = W1 @ x^T, +b1 via a ScalarE Identity copy) pipeline
    behind its own DMA, so the DVE starts ~16us instead of waiting
    ~45us for the full 6.4MB x DMA.
  - Layer-1 LIF is the fused custom DVE op (1 elem/lane/cycle @0.96GHz —
    the DVE is the pacing engine; its total work is the runtime wall).
    Membrane/sign tiles are full-width per STEP, written in per-block
    slices during the fill phase, which is emitted as one consecutive
    RUN per block (the engines' 4-deep wait queues then let each block's
    serial chain execute at its own ~0.5-1.1us/step latency as soon as
    its cur1 lands, instead of head-of-line blocking on later blocks).
    All layer-2 work rides with the last block's run. From step MERGE_T
    on, all blocks have caught up and each step is ONE full-width LIF +
    ONE full-width Sign at ~2.43us/step, the DVE roofline.
  - Spikes are Sign(m-1) (+-1 bf16) on the Scalar engine; layer-2 chunk
    matmuls use 0.5*W2 hi/lo bf16 moving + a K=1 constant-row matmul
    (exact to ~1e-7). Layer-2 LIF ([128,160]) is one lagged custom DVE op
    reading cur2 from PSUM; only mem2 is DMA'd out and the host
    recomputes spk2 = (mem2 > 1) bit-exactly.
"""
import numpy as np
import ml_dtypes

import concourse.bacc as bacc
import concourse.mybir as mybir
import concourse.tile as tile
from concourse.tile import add_dep_helper
import concourse.dve_ops as dve_ops
from concourse.dve_spec import Spec, Src0, Src1, C0, C1
from concourse.dve_ops import DveOp
from concourse.bass_utils import run_bass_kernel_spmd

F32 = mybir.dt.float32
BF16 = mybir.dt.bfloat16
Alu = mybir.AluOpType

N_CORES = 8
B_FULL = 16384
B = B_FULL // N_CORES          # 2048 batch rows per core
D_IN = 784                     # 28*28
H1 = 128
H2 = 10
STEPS = 25
KT = 112                       # K-tile size: 784 = 7 * 112
NKT = D_IN // KT
NCHUNK = B // 128              # 16 batch chunks per core
THRESH = 1.0

# Column-block ladder (name, col0, width): small first block -> early start.
BLOCKS = [
    ("A", 0,    256),
    ("B", 256,  512),
    ("C", 768,  512),
    ("D", 1280, 768),
]
DMA_ORDER = ["A", "B", "C", "D"]
MERGE_T = 10      # steps >= this use single full-width LIF/Sign ops
LAG = 4           # lif2(s) trails the step-s chunk matmuls by 4 steps


def _register_lif():
    """Custom DVE op: out = ((in0*s0 + in1) - (in0 > s1))."""
    if "LIF_STEP_ANT" in dve_ops._SUB_OPCODE_FOR_NAME:
        return next(op for op in dve_ops.OPS if op.name == "LIF_STEP_ANT")
    op = DveOp(
        "LIF_STEP_ANT",
        Spec(
            body=(Src0 * C0 + Src1) - (Src0 > C1),
            reference=lambda in0, in1, s0, s1, imm2: (
                (in0 * s0 + in1) - (in0 > s1).astype(np.float32)
            ),
        ),
        subdim=False,
        uops_sha={"v3": "4d971942aba05d49", "v4": "da6677450a1cb1b9"},
    )
    dve_ops.OPS.append(op)
    dve_ops._SUB_OPCODE_FOR_NAME[op.name] = (
        dve_ops._CUSTOM_DVE_ROW_BASE + len(dve_ops.OPS) - 1
    )
    dve_ops.CUSTOM_DVE_SPECS[op.name] = op.spec
    return op


_GRAPH_CACHE = {}


def _build_graph(beta1: float, beta2: float):
    key = (beta1, beta2)
    if key in _GRAPH_CACHE:
        return _GRAPH_CACHE[key]
    LIF = _register_lif()
    Sign = mybir.ActivationFunctionType.Sign
    Ident = mybir.ActivationFunctionType.Identity

    nc = bacc.Bacc("TRN2", target_bir_lowering=False, debug=False,
                   num_devices=N_CORES)

    xt_d = nc.dram_tensor("xt", [NKT, KT, B], F32, kind="ExternalInput").ap()
    w1t_d = nc.dram_tensor("w1t", [KT, NKT * H1], F32, kind="ExternalInput").ap()
    b1_d = nc.dram_tensor("b1", [H1, 1], F32, kind="ExternalInput").ap()
    w2h_d = nc.dram_tensor("w2h", [H1, H2], BF16, kind="ExternalInput").ap()
    w2l_d = nc.dram_tensor("w2l", [H1, H2], BF16, kind="ExternalInput").ap()
    cc_d = nc.dram_tensor("cc160", [1, NCHUNK * H2], F32, kind="ExternalInput").ap()

    out_mem = nc.dram_tensor("out_mem", [STEPS, 128, NCHUNK * H2], F32,
                             kind="ExternalOutput").ap()

    with tile.TileContext(nc) as tc:
        with tc.tile_pool(name="const", bufs=1) as cpool, \
             tc.tile_pool(name="xin", bufs=1) as xpool, \
             tc.tile_pool(name="m1p", bufs=10) as m1pool, \
             tc.tile_pool(name="m2p", bufs=12) as m2pool, \
             tc.tile_pool(name="sgp", bufs=11) as sgpool, \
             tc.tile_pool(name="psw", bufs=1, space="PSUM") as pswarm, \
             tc.tile_pool(name="ps", bufs=7, space="PSUM") as pspool:

            # preload ACT tables (Sign + Identity) before anything else
            warm_t = cpool.tile([H1, 1], F32, tag="warm")
            nc.scalar.activation(warm_t[:], nc.const_aps.tensor(0.0, (H1, 1)),
                                 Sign, bias=0.0)
            nc.scalar.activation(warm_t[:], nc.const_aps.tensor(0.0, (H1, 1)),
                                 Ident, bias=0.0)

            # ---- DMA order: w1t+b1 (needed first), A/B xt, the cold
            # constants (first used ~25us in), then C/D xt ----
            ones_t = cpool.tile([1, H1], F32, tag="ones")
            nc.vector.memset(ones_t[:], 1.0)
            neg1_t = cpool.tile([H1, 1], F32, tag="neg1")
            nc.vector.memset(neg1_t[:], -1.0)
            zeros_t = cpool.tile([KT, 512], F32, tag="zeros")
            nc.vector.memset(zeros_t[:], 0.0)
            w1t_all = cpool.tile([KT, NKT * H1], F32, tag="w1t")
            nc.sync.dma_start(w1t_all[:], w1t_d)
            w1t_tiles = [w1t_all[:, k * H1:(k + 1) * H1] for k in range(NKT)]
            b1_t = cpool.tile([H1, 1], F32, tag="b1")
            nc.sync.dma_start(b1_t[:], b1_d)

            # xt tiles: rotating pools shared by (A,C) and (B,D). C's DMA
            # reuses A's buffers so it naturally waits for A's matmul
            # readers (done long before C's data could arrive anyway), and
            # likewise D after B — sequencing the stream without explicit
            # completion-semaphore chains. A and B stream concurrently from
            # the start; all DMAs go down the single Sync queue.
            blkmap = {bn: (c0, w) for (bn, c0, w) in BLOCKS}
            PAIR_W = {"A": 512, "C": 512, "B": 768, "D": 768}
            PAIR_TAG = {"A": "ac", "C": "ac", "B": "bd", "D": "bd"}
            xt_tiles = {}

            def emit_xt(bn):
                c0, w = blkmap[bn]
                for k in range(NKT):
                    xt = xpool.tile([KT, PAIR_W[bn]], F32,
                                    tag=f"xt_{PAIR_TAG[bn]}{k}",
                                    name=f"xt_{bn}{k}")[:, :w]
                    xt_tiles[(bn, k)] = xt
                    nc.sync.dma_start(xt[:], xt_d[k][:, c0:c0 + w])

            emit_xt("A")
            emit_xt("B")
            w2h_t = cpool.tile([H1, H2], BF16, tag="w2h")
            nc.sync.dma_start(w2h_t[:], w2h_d)
            w2l_t = cpool.tile([H1, H2], BF16, tag="w2l")
            nc.sync.dma_start(w2l_t[:], w2l_d)
            cc_t = cpool.tile([1, NCHUNK * H2], F32, tag="cc160")
            nc.sync.dma_start(cc_t[:], cc_d)
            emit_xt("C")
            emit_xt("D")

            # ---- cur1 per block: 7 fp32 K-matmuls -> psum -> sbuf(+b1) ----
            # HAM warm-up: zero matmuls into block A's psum while its DMA
            # streams, so the PE is at full clock for the real fp32 work.
            cur1_t = cpool.tile([H1, B], F32, tag="cur1")
            ps_warm = pswarm.tile([128, 512], F32, tag="psw", name="ps_warm")

            def fillers(n, w=512):
                # Zero matmuls with no data deps: keep the PE's HAM clock
                # ramped while it would otherwise idle waiting for the next
                # block's DMA (a cold PE runs fp32 matmuls at half speed).
                for _ in range(n):
                    nc.tensor.matmul(ps_warm[:, :w], zeros_t[:, :128],
                                     zeros_t[:, :w], start=True, stop=True)

            copy_jobs = {}   # block name -> list of (psum_ap, cur1_slice)
            FILLERS = {"A": 0, "B": 0, "C": 0, "D": 0}
            # short pre-ramp that finishes before block A's data lands
            fillers(6, w=256)
            for bn in DMA_ORDER:   # PE queue order must match DMA arrival
                c0, w = blkmap[bn]
                fillers(FILLERS[bn])
                jobs = []
                off = c0
                rem = w
                while rem > 0:
                    pw = min(rem, 512)
                    ps = pspool.tile([128, 512], F32, tag="ps",
                                     name=f"cur1_{bn}_{off}")[:, :pw]
                    for k in range(NKT):
                        nc.tensor.matmul(ps, w1t_tiles[k],
                                         xt_tiles[(bn, k)][:, off - c0:
                                                           off - c0 + pw],
                                         start=(k == 0), stop=(k == NKT - 1))
                    jobs.append((ps, cur1_t[:, off:off + pw]))
                    off += pw
                    rem -= pw
                copy_jobs[bn] = jobs

            def emit_copy(bn):
                for ps, dst in copy_jobs[bn]:
                    # psum -> sbuf with +b1 per-partition bias on ScalarE
                    nc.scalar.activation(dst, ps, Ident, bias=b1_t[:])


            # ---- state: full-width per-step tiles, slice-written ----
            m1_tiles = {}       # t -> [128, 2048] membrane tile
            sg_tiles = {}       # t -> [128, 2048] bf16 sign tile
            m2_cur = None       # layer-2 membrane [128, 160]
            ps2 = {}            # s -> psum slice [128, 160]
            nmm = {}            # s -> chunk-matmul pairs emitted so far

            def m1_in(t, c0, w):
                # membrane state entering step t's update (t>=1): step t-1's
                # output; step 0's "update" is free (m1(0) = cur1).
                if t == 1:
                    return cur1_t[:, c0:c0 + w]
                return m1_tiles[t - 1][:, c0:c0 + w]

            def get_m1(t):
                if t not in m1_tiles:
                    m1_tiles[t] = m1pool.tile([H1, B], F32, tag="m1",
                                              name=f"m1_{t}")
                return m1_tiles[t]

            def get_sg(t):
                if t not in sg_tiles:
                    sg_tiles[t] = sgpool.tile([H1, B], BF16, tag="sg",
                                              name=f"sg_{t}")
                return sg_tiles[t]

            def emit_lif1(t, c0, w):
                out = get_m1(t)[:, c0:c0 + w]
                nc.vector._custom_dve(LIF, out=out, in0=m1_in(t, c0, w),
                                      in1=cur1_t[:, c0:c0 + w],
                                      s0=beta1, s1=THRESH)

            def emit_sign(t, c0, w):
                src = cur1_t[:, c0:c0 + w] if t == 0 else \
                    m1_tiles[t][:, c0:c0 + w]
                nc.scalar.activation(get_sg(t)[:, c0:c0 + w], src, Sign,
                                     bias=neg1_t[:])

            def emit_chunks(s, c0, w):
                # cc row starts the psum accumulation group for step s
                if s not in ps2:
                    p = pspool.tile([128, 512], F32, tag="ps",
                                    name=f"ps2_{s}")[:, :NCHUNK * H2]
                    nc.tensor.matmul(p, ones_t[:], cc_t[:], start=True,
                                     stop=False)
                    ps2[s] = p
                    nmm[s] = 0
                p = ps2[s]
                sgt = sg_tiles[s]
                for c in range(c0 // 128, (c0 + w) // 128):
                    o = p[:, c * H2:(c + 1) * H2]
                    sgc = sgt[:, c * 128:(c + 1) * 128]
                    nmm[s] += 1
                    nc.tensor.matmul(o, sgc, w2h_t[:], start=False, stop=False)
                    nc.tensor.matmul(o, sgc, w2l_t[:], start=False,
                                     stop=(nmm[s] == NCHUNK))

            def dve_lif2(s):
                nonlocal m2_cur
                p = ps2.pop(s)
                m2 = m2pool.tile([128, NCHUNK * H2], F32, tag="m2")
                if s == 0:
                    # m2(0) = cur2(0): beta*0 + cur2 - 0
                    nc.vector.tensor_scalar(m2[:], p, 0.0, None, Alu.add)
                else:
                    nc.vector._custom_dve(LIF, out=m2[:], in0=m2_cur[:],
                                          in1=p, s0=beta2, s1=THRESH)
                m2_cur = m2
                nc.sync.dma_start(out_mem[s], m2[:])

            # PE clock pre-ramp for the chunk-matmul stream: these run in
            # the natural PE idle window after D's cur1 matmuls, before the
            # first sign-gated chunk matmuls can execute.
            fillers(8)

            # ---- fill phase: one consecutive run per block ----
            # Each engine's 4-deep wait-queue lets a block's serial chain
            # execute at its own latency (~0.73us/step) as soon as its
            # cur1 lands; runs are ordered by DMA arrival so nothing parks
            # more than one instruction deep. All layer-2 work (chunk
            # matmuls, cc row, lagged lif2) rides with the LAST block's
            # run, when every block's signs for that step already exist.
            # Each block's psum->sbuf copy is emitted mid-way through the
            # PREVIOUS block's run, so it executes the moment its psum is
            # ready instead of queueing behind nine sign activations.
            for bn in DMA_ORDER:
                c0, w = blkmap[bn]
                if bn == "C":
                    # copy(D) right after copy(C), ahead of C's signs: the
                    # D-run's LIF chain then starts as soon as D's psum is
                    # ready (~32us) instead of queueing behind nine signs.
                    emit_copy("C")
                    emit_copy("D")
                elif bn == "A":
                    emit_copy(bn)
                for t in range(MERGE_T):
                    if t >= 1:
                        emit_lif1(t, c0, w)
                    emit_sign(t, c0, w)
                    if bn == "A" and t == 7:
                        # copy(B) ahead of A's last signs: it then runs the
                        # moment B's psum is ready (~21us), not after them
                        emit_copy("B")
                    if bn == DMA_ORDER[-1]:
                        emit_chunks(t, 0, B)
                        if t >= LAG:
                            dve_lif2(t - LAG)
            # ---- merged phase: one full-width LIF + Sign per step ----
            for t in range(MERGE_T, STEPS - 1):
                emit_lif1(t, 0, B)
                emit_sign(t, 0, B)
                emit_chunks(t, 0, B)
                dve_lif2(t - LAG)
            # final step: 4 column-group pipelines (lif/sign/chunks overlap)
            # so the drain chain is ~one group long instead of full-width
            t = STEPS - 1
            for s in range(STEPS - 1 - LAG, STEPS - 1):
                dve_lif2(s)   # overlap these with the final step's signs
            for g0 in range(0, B, 512):
                emit_lif1(t, g0, 512)
                emit_sign(t, g0, 512)
                emit_chunks(t, g0, 512)
            dve_lif2(STEPS - 1)

    nc.compile()
    _GRAPH_CACHE[key] = nc
    return nc


def prepare_in_maps(x, W1, b1, W2, b2):
    x = np.asarray(x, dtype=np.float32)
    W1 = np.asarray(W1, dtype=np.float32)
    b1 = np.asarray(b1, dtype=np.float32)
    W2 = np.asarray(W2, dtype=np.float32)
    b2 = np.asarray(b2, dtype=np.float32)
    xf = x.reshape(B_FULL, D_IN)
    xT = xf.T.reshape(NKT, KT, B_FULL)                    # [7, 112, 16384]
    W1T = np.ascontiguousarray(
        W1.T.reshape(NKT, KT, H1).transpose(1, 0, 2).reshape(KT, NKT * H1))
    b1c = np.ascontiguousarray(b1.reshape(H1, 1))
    W2T_half = 0.5 * W2.T                                 # [128, 10]
    w2h = W2T_half.astype(ml_dtypes.bfloat16)
    w2l = (W2T_half - w2h.astype(np.float32)).astype(ml_dtypes.bfloat16)
    ccrow = (0.5 * W2.sum(axis=1) + b2).astype(np.float32)
    cc160 = np.ascontiguousarray(
        np.tile(ccrow, NCHUNK).reshape(1, NCHUNK * H2).astype(np.float32))
    in_maps = []
    for i in range(N_CORES):
        shard = np.ascontiguousarray(xT[:, :, i * B:(i + 1) * B])
        in_maps.append({
            "xt": shard, "w1t": W1T, "b1": b1c,
            "w2h": w2h, "w2l": w2l, "cc160": cc160,
        })
    return in_maps


def kernel(x, W1, b1, W2, b2, beta1, beta2):
    bb1 = float(np.clip(np.float32(beta1), 0.0, 1.0))
    bb2 = float(np.clip(np.float32(beta2), 0.0, 1.0))
    in_maps = prepare_in_maps(x, W1, b1, W2, b2)
    nc = _build_graph(bb1, bb2)
    res = run_bass_kernel_spmd(nc, in_maps, list(range(N_CORES)), trace=False)

    mem_parts = []
    for i in range(N_CORES):
        r = res.results[i]
        # [25, 128, 16*10] -> [25, 2048, 10]; batch = chunk*128 + partition
        mem = r["out_mem"].reshape(STEPS, 128, NCHUNK, H2)
        mem_parts.append(np.transpose(mem, (0, 2, 1, 3)).reshape(STEPS, B, H2))
    mem2 = np.ascontiguousarray(
        np.concatenate(mem_parts, axis=1).astype(np.float32))
    # spikes are a pure function of the (bit-exact) membrane values
    spk2 = (mem2 > np.float32(THRESH)).astype(np.float32)
    return spk2, mem2
